# revision 67
# baseline (speedup 1.0000x reference)
"""CRF loss kernel for Trainium2 (8 NeuronCores, sequence-parallel).

reference: mean_b( logZ_b - score_b ) for a linear-chain CRF with
B=256, S=512, T=128.

The forward algorithm's 511-step recurrence u_s = (A^T u_{s-1}) o e_s
is latency-bound on device (~0.9 us per step of matmul + DVE multiply
incl. PSUM drain/ack and semaphores), so instead of data-parallel batch
sharding this kernel shards the SEQUENCE: products of positive random
matrices forget their starting direction at ~e^-1.9/step (measured), so
each core computes one ~64-step segment for ALL 256 batches starting
from the uniform direction, and the host telescopes

    logZ_b = sum_c log r_cb - 7*log(T) + 511*kappa + offsets

where r_c = 1^T M_seg_c 1^ is the bilinear segment value, computed
meet-in-the-middle: a fwd chain from the uniform direction and a bwd
chain from the exact ones (resp. exp(end)) row vector, meeting after 32
steps each. The direction error at the 7 internal boundaries is below
the arithmetic noise. Serial depth per core: 32 rounds (vs 256 for
batch-parallel meet-in-the-middle), each round = fwd step + bwd step at
width 256, anti-phased so TensorE and VectorE ping-pong between the two
chains (round ~887 ns, within 6% of the DVE-busy floor).

kappa (exact per-step log growth of batch 0, host fp64) is split
between the bf16 stationaries A*e^-(kappa-1) (exact rescale) and the
emissions e^(em-1), so the state growth is ~zero. Emissions are
pre-exponentiated on the host and shipped as fp8 e4m3 multipliers
(64 KB per round; bf16 would be DMA-feed-bound), fwd on the Sync DMA
queue and bwd on the Scalar queue. Both chains start from an on-device
memset ones state; core 0's u_0 = exp(start + em_0) rides in its slot-0
emission behind an identity stationary, and core 7's exp(end) closure
is folded into its first bwd emission, so a single uniform program runs
on all 8 cores. fp8 + boundary truncation cost ~4e-1 absolute on a
~3000 logZ (1e-4 relative; the gate is 2e-2).

Numerator (score of the tagged path, ~0.1% of FLOPs) on host in fp64.
"""

import numpy as np
import ml_dtypes

B, S, T = 256, 512, 128
NCORES = 8
NF = NB = 32               # fwd/bwd slots per core
EMOFF = 1.0                # log offset folded into the fp8 emissions
U0OFF = 1.5                # extra offset on core 0's u_0 loader (fp8 range)
ENDOFF = 2.0               # extra offset on core 7's end closure (fp8 range)

_nc_cache = None
LAST_RESULTS = None
DEVICE_OK = None           # False when the host fallback produced the result


def _plans(c):
    """Per-core fwd/bwd slot lists: (stationary, emission_spec).

    stationary: 'A' (=exp(trans), fwd), 'T' (=exp(trans).T, bwd),
    'I' (identity pad). emission_spec: ('step', s) | ('u0',) |
    ('end511',).
    Zero-warmup split (direction error at the 7 internal boundaries is
    dominated by the fp8 quantization noise): core 0: steps 1..63,
    cores 1..6: 64c..64c+63, core 7: 448..511. Core 0's slot 0 doubles
    as the u_0 loader (identity stationary x ones init); core 7's
    exp(end) closure is folded into its first bwd emission together
    with an e^-2 rescale (fp8 range), added back by the host.
    """
    if c == 0:
        f = [('A', ('u0',))] + [('A', ('step', s)) for s in range(1, 32)]
        b = [('T', ('step', s)) for s in range(63, 31, -1)]
    elif c < 7:
        f = [('A', ('step', s)) for s in range(64 * c, 64 * c + 32)]
        b = [('T', ('step', s)) for s in range(64 * c + 63, 64 * c + 31, -1)]
    else:
        f = [('A', ('step', s)) for s in range(448, 480)]
        b = [('T', ('end511',))] + [('T', ('step', s))
                                    for s in range(510, 479, -1)]
    assert len(f) == NF and len(b) == NB
    return f, b


def _build_nc():
    import concourse.bacc as bacc
    import concourse.mybir as mybir
    import concourse.tile as tile

    fp32 = mybir.dt.float32
    bf16 = mybir.dt.bfloat16
    f8 = mybir.dt.float8e4
    mult = mybir.AluOpType.mult

    nc = bacc.Bacc("TRN2", target_bir_lowering=False, debug=False)

    emf = nc.dram_tensor("emf", [T, NF, B], f8, kind="ExternalInput")
    emb = nc.dram_tensor("emb", [T, NB, B], f8, kind="ExternalInput")
    bootf = nc.dram_tensor("bootf", [T, T], bf16, kind="ExternalInput")
    bootb = nc.dram_tensor("bootb", [T, T], bf16, kind="ExternalInput")
    outr = nc.dram_tensor("outr", [T, B], bf16, kind="ExternalOutput")

    chunks = [(0, 1), (1, 3), (4, 4), (8, 6), (14, 6), (20, 6), (26, 6)]

    with tile.TileContext(nc) as tc:
        with (
            tc.tile_pool(name="const", bufs=1) as constp,
            tc.tile_pool(name="emp", bufs=2 * len(chunks)) as emp,
            tc.tile_pool(name="sbp", bufs=4) as sbp,
            tc.tile_pool(name="vp", bufs=2, space="PSUM") as vp,
            tc.tile_pool(name="bp", bufs=2, space="PSUM") as bp,
        ):
            bf_tile = constp.tile([T, T], bf16)
            bb_tile = constp.tile([T, T], bf16)
            ones_sb = constp.tile([T, B], bf16)
            nc.gpsimd.memset(ones_sb[:], 1.0)

            emf_tiles = {}
            emb_tiles = {}

            def load_chunk(ci, which="fb"):
                s0, ln = chunks[ci]
                if "f" in which:
                    tf = emp.tile([T, ln, B], f8, tag="emf")
                    nc.sync.dma_start(tf[:], emf[:, s0:s0 + ln, :])
                    emf_tiles[ci] = tf
                if "b" in which:
                    tb = emp.tile([T, ln, B], f8, tag="emb")
                    nc.scalar.dma_start(tb[:], emb[:, s0:s0 + ln, :])
                    emb_tiles[ci] = tb

            def em_slice(tiles, k):
                for ci, (s0, ln) in enumerate(chunks):
                    if s0 <= k < s0 + ln:
                        return tiles[ci][:, k - s0, :]
                raise AssertionError(k)

            # fwd feed on the Sync DMA queue, bwd feed in parallel on the
            # Scalar queue (fp8 emissions keep each well under queue rate).
            # emb chunk 0 leads its queue: the first bwd multiply needs no
            # matmul, so early data lets the chains start anti-phased.
            # boots lead their queues (tiny, gate the first matmuls), then
            # a 1-slot lead chunk per direction, then the bulk
            nc.sync.dma_start(bf_tile[:], bootf[:])
            nc.scalar.dma_start(bb_tile[:], bootb[:])
            for ci in range(len(chunks)):
                load_chunk(ci)

            fa_ap = bf_tile[:, 0:T]
            bat_ap = bb_tile[:, 0:T]

            u = ones_sb[:]
            z_prev = ones_sb[:]
            zp = None
            for k in range(NF):
                # fwd slot k: u <- (stat^T u) o e_f[k]
                vf = vp.tile([T, B], fp32, tag="vf")
                nc.tensor.matmul(vf[:], fa_ap, u, start=True, stop=True)
                # bwd multiply leads the DVE queue: it only needs the
                # previous round's matmul output
                tmp = sbp.tile([T, B], bf16, tag="w")
                zsrc = z_prev if zp is None else zp[:]
                nc.vector.tensor_tensor(tmp[:], zsrc,
                                        em_slice(emb_tiles, k), mult)
                u_new = sbp.tile([T, B], bf16, tag="u")
                nc.vector.tensor_tensor(u_new[:], vf[:],
                                        em_slice(emf_tiles, k), mult)
                u = u_new[:]
                u_tile = u_new
                zp = bp.tile([T, B], fp32, tag="vb")
                nc.tensor.matmul(zp[:], bat_ap, tmp[:],
                                 start=True, stop=True)

            # meet: r_c = sum_t z[t,b] * u[t,b]; host does the t-reduction.
            # Column-split so each half's DMA (on its own queue) can start
            # as soon as that half of the product is ready.
            prod = sbp.tile([T, B], bf16, tag="u")
            h = B // 2
            nc.vector.tensor_tensor(prod[:, 0:h], zp[:, 0:h],
                                    u_tile[:, 0:h], mult)
            nc.sync.dma_start(outr[:, 0:h], prod[:, 0:h])
            nc.vector.tensor_tensor(prod[:, h:B], zp[:, h:B],
                                    u_tile[:, h:B], mult)
            nc.scalar.dma_start(outr[:, h:B], prod[:, h:B])

    nc.compile()
    return nc


def _get_nc():
    global _nc_cache
    if _nc_cache is None:
        _nc_cache = _build_nc()
    return _nc_cache


def _ensure_ntff_hook_importable():
    """bass_utils imports antenv.axon_hooks when BASS_TRACE is set; this
    image's antenv package lacks that module, so provide a shim rather
    than crash (and enable profiling when the axon .so supports it)."""
    import sys
    import types
    try:
        import antenv.axon_hooks  # noqa: F401
        return
    except ImportError:
        pass
    try:
        import antenv
        from trn_agent_boot.trn_boot import _ntff_profile_via_ctypes
        hook = _ntff_profile_via_ctypes('/opt/axon/libaxon_pjrt.so')
    except Exception:
        try:
            import antenv
        except ImportError:
            return
        hook = None
    mod = types.ModuleType("antenv.axon_hooks")
    mod._hook = hook
    mod.get_axon_ntff_profile_hook = lambda: mod._hook
    mod.set_axon_ntff_profile_hook = lambda h: setattr(mod, "_hook", h)
    antenv.axon_hooks = mod
    sys.modules["antenv.axon_hooks"] = mod


def _kappa_host(em, trans, start):
    """Exact per-step log-mass growth of batch 0 (fp64 log-space forward)."""
    sc = start.astype(np.float64) + em[0, 0].astype(np.float64)
    t64 = trans.astype(np.float64)
    for i in range(1, em.shape[1]):
        x = sc[:, None] + t64 + em[0, i].astype(np.float64)[None, :]
        mx = x.max(axis=0)
        sc = mx + np.log(np.exp(x - mx[None, :]).sum(axis=0))
    mx = sc.max()
    return float((mx + np.log(np.exp(sc - mx).sum())) / (em.shape[1] - 1))


def _numerator_host(em, tags, mask, trans, start, end):
    em64 = em.astype(np.float64)
    tags = tags.astype(np.int64)
    bidx = np.arange(em.shape[0])
    score = start.astype(np.float64)[tags[:, 0]] + em64[bidx, 0, tags[:, 0]]
    trans_term = trans.astype(np.float64)[tags[:, 1:], tags[:, :-1]]
    em_term = np.take_along_axis(em64[:, 1:], tags[:, 1:, None], axis=2)[..., 0]
    m = mask[:, 1:].astype(np.float64)
    score = score + ((trans_term + em_term) * m).sum(axis=1)
    last_idx = mask.sum(axis=1).astype(np.int64) - 1
    last_tags = np.take_along_axis(tags, last_idx[:, None], axis=1)[:, 0]
    return score + end.astype(np.float64)[last_tags]


def _reference_host(em, tags, mask, trans, start, end):
    """Pure-numpy fp64 fallback (exact semantics incl. arbitrary masks)."""
    em64 = em.astype(np.float64)
    score = start.astype(np.float64) + em64[:, 0]  # [B, T]
    t64 = trans.astype(np.float64)
    for i in range(1, em.shape[1]):
        x = score[:, :, None] + t64[None] + em64[:, i][:, None, :]
        mx = x.max(axis=1)
        nxt = mx + np.log(np.exp(x - mx[:, None, :]).sum(axis=1))
        score = np.where(mask[:, i][:, None], nxt, score)
    x = score + end.astype(np.float64)
    mx = x.max(axis=1, keepdims=True)
    denom = (mx[:, 0] + np.log(np.exp(x - mx).sum(axis=1)))
    numer = _numerator_host(em, tags, mask, trans, start, end)
    return np.float32((denom - numer).mean())


def kernel(**inputs):
    global LAST_RESULTS, DEVICE_OK
    DEVICE_OK = False
    em = np.asarray(inputs["emissions"], dtype=np.float32)
    tags = np.asarray(inputs["tags"])
    mask = np.asarray(inputs["mask"])
    trans = np.asarray(inputs["transitions"], dtype=np.float32)
    start = np.asarray(inputs["start_transitions"], dtype=np.float32)
    end = np.asarray(inputs["end_transitions"], dtype=np.float32)

    if not mask.all():
        # device scan assumes a dense mask (guaranteed by the input spec);
        # fall back to the exact host path otherwise
        return _reference_host(em, tags, mask, trans, start, end)

    _ensure_ntff_hook_importable()
    from concourse.bass_utils import run_bass_kernel_spmd

    nc = _get_nc()
    kap = _kappa_host(em, trans, start)
    # per-step rescale e^-kap split between the fp8 emissions (e^-EMOFF,
    # keeps multipliers ~O(1) for e4m3) and the bf16 stationaries
    # (e^-(kap-EMOFF), exact in bf16) so the state growth is ~zero
    aoff = np.float32(kap - EMOFF)
    bf = ml_dtypes.bfloat16
    f8 = ml_dtypes.float8_e4m3fn

    # exp-space emission multipliers, [T, S, B] fp8
    ex = np.exp(em.transpose(2, 1, 0) - np.float32(EMOFF)).astype(f8)
    a_exp = np.exp(trans - aoff).astype(bf)
    at_exp = np.ascontiguousarray(a_exp.T)
    # core 0 slot 0 runs the regular stationary on the ones state, so its
    # "emission" u0 / (A_sc^T 1) makes the slot output exactly u0*e^-U0OFF
    colsum = a_exp.astype(np.float64).sum(axis=0)               # A_sc^T 1
    u0 = np.ascontiguousarray(
        (np.exp(start[None, :] + em[:, 0, :] - np.float32(U0OFF))
         / colsum[None, :]).T.astype(f8))                       # [T, B]
    end511 = np.ascontiguousarray(
        (np.exp(em[:, 511, :].T - np.float32(EMOFF))
         * np.exp(end - np.float32(ENDOFF))[:, None]).astype(f8))

    def emtile(spec):
        kind = spec[0]
        if kind == 'u0':
            return u0[:, None, :]
        if kind == 'end511':
            return end511[:, None, :]
        return ex[:, spec[1]:spec[1] + 1, :]

    def gather(slots):
        return np.ascontiguousarray(
            np.concatenate([emtile(sp) for _, sp in slots], axis=1))

    in_maps = []
    spans = np.zeros(NCORES)
    for c in range(NCORES):
        f, b = _plans(c)
        spans[c] = sum(sp[0] != 'u0' for _, sp in f) + len(b)
        in_maps.append({
            "emf": gather(f),
            "emb": gather(b),
            "bootf": a_exp,
            "bootb": at_exp,
        })
    assert spans.sum() == 511

    LAST_RESULTS = run_bass_kernel_spmd(nc, in_maps, list(range(NCORES)))
    r = np.stack([LAST_RESULTS.results[c]["outr"] for c in range(NCORES)])
    r = r.astype(np.float64).sum(axis=1)    # [8, B]

    if not (np.isfinite(r).all() and (r > 0).all()):
        return _reference_host(em, tags, mask, trans, start, end)
    DEVICE_OK = True

    # n_c = 1^T (ones init) = T for the 7 internal boundaries; U0OFF/ENDOFF
    # undo the loader/closure rescales
    logz = (np.log(r).sum(axis=0) - 7 * np.log(T)
            + kap * spans.sum() + U0OFF + ENDOFF)
    numer = _numerator_host(em, tags, mask, trans, start, end)
    return np.float32((logz - numer).mean())
